# revision 9
# baseline (speedup 1.0000x reference)
"""Trainium2 Bass kernel for nn_Attn_30623116820602.

Low-rank-projected causal multi-head attention:
  q/k/v = (x @ A) @ B  (rank 192), RoPE on q,k, causal softmax attention,
  output projection.  x: [128, 256, 768] fp32.

Sharding: pure data-parallel over batch (16 items per core, 8 cores).
Feature-major layout on device (d_model on partitions); host pre-transposes
x per core and post-transposes the result.

v4: bf16 matmul operands everywhere (psum stays f32).
  - RoPE emitted per head-PAIR on [128,1024] tiles: psum->sbuf copies on
    ScalarE, rotate-half partition swap via two Sync-queue DMAs, the two
    multiplies on DVE (bf16 2x mode), the add on GpSimd.
  - Causal mask: exp runs unmasked; the two causal triangles of each E tile
    are then zeroed by one strided DVE multiply per (b,g) against a resident
    0/1 triangle table.  (Cheaper than the old identity-stationary -1e4
    matmul: N<=256 matmuls are LDWEIGHTS-bound on this stack.)
  - Both heads' exp outputs share one E2 [128,768] tile, so the softmax
    denominators take two wide strided matmuls instead of four narrow ones.
  - Denominators broadcast across partitions by an all-ones stationary;
    normalization fused into the PSUM->SBUF move of the attention output.
"""

import math
import sys

sys.path.insert(0, "/opt/trn_rl_repo")

import numpy as np
import ml_dtypes


def _to_bf16(a):
    return a.astype(ml_dtypes.bfloat16)


B, T, D = 128, 256, 768
H, HD = 6, 128
RANK = 192  # padded to 256 on device
N_CORES = 8
B_LOC = B // N_CORES  # 16
N_PAIRS = B_LOC // 2  # 8 (2 batch items per pipeline iteration)
SCALE = 1.0 / math.sqrt(HD)

_CACHE = {}


def build_program(n_pairs=N_PAIRS):
    import concourse.tile as tile
    from concourse import bacc, mybir
    from contextlib import ExitStack

    f32 = mybir.dt.float32
    bf16 = mybir.dt.bfloat16
    TOK = n_pairs * 512

    nc = bacc.Bacc("TRN2", target_bir_lowering=False, debug=False,
                   num_devices=N_CORES)

    def din(name, shape, dt=bf16):
        return nc.dram_tensor(name, shape, dt, kind="ExternalInput").ap()

    xT = din("xT", [6, 128, TOK])
    qA_l, kA_l, vA_l = (din(n, [6, 128, 192]) for n in ("qA_l", "kA_l", "vA_l"))
    qkAh_l = din("qkAh_l", [6, 128, 128])  # [qA_hi | kA_hi] fused tails
    qB_l, kB_l, vB_l = (din(n, [2, 128, 768]) for n in ("qB_l", "kB_l", "vB_l"))
    ow_l = din("ow_l", [6, 128, 768])
    cos4 = din("cos4", [128, 1024])
    sin4 = din("sin4", [128, 1024])
    tri_m = din("tri_m", [128, 256])     # 0 / -1e4 triangle pair
    eye_m = din("eye_m", [128, 128])     # identity
    ones_m = din("ones_m", [128, 128])   # all ones
    outT = nc.dram_tensor("outT", [6, 128, TOK], bf16,
                          kind="ExternalOutput").ap()

    with tile.TileContext(nc) as tc:
        with ExitStack() as ctx:
            wp = ctx.enter_context(tc.tile_pool(name="w", bufs=1))
            xp = ctx.enter_context(tc.tile_pool(name="xt", bufs=2))
            xrp = ctx.enter_context(tc.tile_pool(name="xr", bufs=2))
            mp = ctx.enter_context(tc.tile_pool(name="msb", bufs=3))
            qkp = ctx.enter_context(tc.tile_pool(name="qk", bufs=2))
            ep = ctx.enter_context(tc.tile_pool(name="eexp", bufs=6))
            rp = ctx.enter_context(tc.tile_pool(name="rec", bufs=3))
            aop = ctx.enter_context(tc.tile_pool(name="ao", bufs=2))
            fp = ctx.enter_context(tc.tile_pool(name="fout", bufs=2))
            ps = ctx.enter_context(tc.tile_pool(name="ps", bufs=4, space="PSUM"))
            pst = ctx.enter_context(tc.tile_pool(name="pst", bufs=2, space="PSUM"))
            psb = ctx.enter_context(tc.tile_pool(name="psb", bufs=2, space="PSUM"))

            # ---- resident weights / constants (all bf16) ----
            def wload(name, src, shape, perm=None):
                t = wp.tile(shape, bf16, tag=name, name=name)
                s = src if perm is None else src.rearrange(perm)
                nc.gpsimd.dma_start(t[:], s)
                return t

            qA_s = wload("qA", qA_l, [128, 6, 192], "k p m -> p k m")
            kA_s = wload("kA", kA_l, [128, 6, 192], "k p m -> p k m")
            vA_s = wload("vA", vA_l, [128, 6, 192], "k p m -> p k m")
            qkAh_s = wload("qkAh", qkAh_l, [128, 6, 128], "k p m -> p k m")
            qB_s = wload("qB", qB_l, [128, 2, 768], "k p m -> p k m")
            kB_s = wload("kB", kB_l, [128, 2, 768], "k p m -> p k m")
            vB_s = wload("vB", vB_l, [128, 2, 768], "k p m -> p k m")
            ow_s = wload("ow", ow_l, [128, 6, 768], "k p m -> p k m")
            cos_s = wload("cos", cos4, [128, 1024])
            sin_s = wload("sin", sin4, [128, 1024])
            tri_s = wload("tri", tri_m, [128, 256])
            eye_s = wload("eye", eye_m, [128, 128])
            ones_s = wload("ones", ones_m, [128, 128])

            def emit_outproj(aosb_prev, pr_prev, mts, half=None):
                # half=None: both batch items (512 tok); half=b: one item
                w = 512 if half is None else 256
                c0 = 0 if half in (None, 0) else 256
                tokp = slice(pr_prev * 512 + c0, pr_prev * 512 + c0 + w)
                for mt in mts:
                    fps = psb.tile([128, 512], f32, tag="psb", name="fps")
                    for kt in range(6):
                        mov = (aosb_prev[:, kt, :, :] if half is None
                               else aosb_prev[:, kt, half, :])
                        nc.tensor.matmul(
                            fps[:, 0:w],
                            ow_s[:, kt, mt * 128:(mt + 1) * 128],
                            mov,
                            start=(kt == 0), stop=(kt == 5))
                    fout = fp.tile([128, 512], bf16, tag="fout", name="fout")
                    nc.vector.tensor_copy(fout[:, 0:w], fps[:, 0:w])
                    nc.sync.dma_start(outT[mt, :, tokp], fout[:, 0:w])

            def emit_proj1(xt, A_s, pname):
                mm = [psb.tile([128, 512], f32, tag="psb", name="p1"),
                      psb.tile([128, 512], f32, tag="psb", name="p1")]
                for mt in range(2):
                    for kt in range(6):
                        nc.tensor.matmul(
                            mm[mt][:],
                            A_s[:, kt, mt * 64:mt * 64 + 128],
                            xt[:, kt, :],
                            start=(kt == 0), stop=(kt == 5))
                xr = xrp.tile([128, 2, 512], bf16, tag=f"xr_{pname}",
                              name=f"xr_{pname}")
                nc.scalar.copy(xr[:, 0, :], mm[0][:])
                nc.scalar.copy(xr[:, 1, :], mm[1][:])
                return xr

            def emit_proj1_slice(xt, A_s, pname):
                # one 128-col stationary slice -> one [128,512] xr tile
                pm = psb.tile([128, 512], f32, tag="psb", name=f"p1{pname}")
                for kt in range(6):
                    nc.tensor.matmul(
                        pm[:], A_s[:, kt, :], xt[:, kt, :],
                        start=(kt == 0), stop=(kt == 5))
                xr = xrp.tile([128, 512], bf16, tag=f"xr_{pname}",
                              name=f"xr_{pname}")
                nc.scalar.copy(xr[:], pm[:])
                return xr

            def emit_proj2_qk2(g, xr_q, xr_k, qsb, ksb):
                # proj2 + RoPE for heads 2g, 2g+1 of q and k, on [128,1024]
                # head-pair tiles (halves the elementwise instruction count).
                for name, xr, B_s, sb in (("q", xr_q, qB_s, qsb),
                                          ("k", xr_k, kB_s, ksb)):
                    msb = mp.tile([128, 1024], bf16, tag="msb", name="msb")
                    for hh in range(2):
                        h = 2 * g + hh
                        hsl = slice(h * 128, (h + 1) * 128)
                        pm = psb.tile([128, 512], f32, tag="psb", name="pm")
                        for kt in range(2):
                            nc.tensor.matmul(
                                pm[:], B_s[:, kt, hsl], xr[kt][:],
                                start=(kt == 0), stop=(kt == 1))
                        nc.scalar.copy(msb[:, hh * 512:(hh + 1) * 512], pm[:])
                    # rotate-half: partition swap via DMA, aligned DVE
                    # multiplies (bf16 2x), add on GpSimd
                    mrot = mp.tile([128, 1024], bf16, tag="mrot", name="mrot")
                    nc.sync.dma_start(mrot[0:64, :], msb[64:128, :])
                    nc.sync.dma_start(mrot[64:128, :], msb[0:64, :])
                    trot = mp.tile([128, 1024], bf16, tag="trot", name="trot")
                    hs = slice(2 * g * 512, 2 * g * 512 + 1024)
                    nc.vector.tensor_tensor(
                        sb[:, hs], msb[:], cos_s[:], mybir.AluOpType.mult)
                    nc.vector.tensor_tensor(
                        trot[:], mrot[:], sin_s[:], mybir.AluOpType.mult)
                    nc.gpsimd.tensor_tensor(
                        sb[:, hs], sb[:, hs], trot[:], mybir.AluOpType.add)

            def emit_vproj(xrv):
                vsb = qkp.tile([128, 4, 768], bf16, tag="vsb", name="vsb")
                for mt in range(4):
                    for nch in range(2):
                        vp = psb.tile([128, 512], f32, tag="psb", name="vp")
                        for kt in range(2):
                            nc.tensor.matmul(
                                vp[:, 0:384],
                                xrv[:, kt, mt * 128:(mt + 1) * 128],
                                vB_s[:, kt, nch * 384:(nch + 1) * 384],
                                start=(kt == 0), stop=(kt == 1))
                        nc.scalar.copy(vsb[:, mt, nch * 384:(nch + 1) * 384],
                                       vp[:, 0:384])
                return vsb

            def emit_att_scores(qkv, b, g):
                # E2 layout per (b, g): [128 keys, 768]; per head hh half:
                #   cols 0:256   = key-tile 0 vs q 0:256
                #   cols 256:384 = key-tile 1 vs q 128:256
                qsb, ksb, vsb = qkv
                E2 = ep.tile([128, 768], bf16, tag="E", name="E")
                del vsb
                for hh in range(2):
                    h = 2 * g + hh
                    qall = slice(h * 512 + b * 256, h * 512 + b * 256 + 256)
                    qhi = slice(h * 512 + b * 256 + 128,
                                h * 512 + b * 256 + 256)
                    k0 = slice(h * 512 + b * 256, h * 512 + b * 256 + 128)
                    k1 = slice(h * 512 + b * 256 + 128,
                               h * 512 + b * 256 + 256)
                    sp = ps.tile([128, 512], f32, tag="ps", name="sp")
                    nc.tensor.matmul(sp[:, 0:256], ksb[:, k0], qsb[:, qall],
                                     start=True, stop=False,
                                     skip_group_check=True)
                    nc.tensor.matmul(sp[:, 256:384], ksb[:, k1], qsb[:, qhi],
                                     start=False, stop=False,
                                     skip_group_check=True)
                    # one matmul adds -1e4 to both causal triangles
                    # (cols 0:128 and 256:384) via a strided psum AP
                    nc.tensor.matmul(
                        sp[:].rearrange("p (a b) -> p a b", a=4)[:, 0:3:2, :],
                        eye_s[:], tri_s[:],
                        start=False, stop=True, skip_group_check=True)
                    nc.scalar.activation(E2[:, hh * 384:hh * 384 + 384],
                                         sp[:, 0:384],
                                         mybir.ActivationFunctionType.Exp,
                                         scale=SCALE)
                return E2

            def emit_att_tail(qkv, aosb, b, g, E2):
                qsb, ksb, vsb = qkv
                # denominators, broadcast across partitions by the all-ones
                # stationary; two wide strided matmuls over both heads.
                dbc = pst.tile([128, 512], f32, tag="pst", name="dbc")
                for hh in range(2):
                    c = hh * 256
                    e = hh * 384
                    nc.tensor.matmul(dbc[:, c:c + 256], ones_s[:],
                                     E2[:, e:e + 256],
                                     start=(hh == 0), stop=False,
                                     skip_group_check=True)
                    nc.tensor.matmul(dbc[:, c + 128:c + 256], ones_s[:],
                                     E2[:, e + 256:e + 384], start=False,
                                     stop=(hh == 1), skip_group_check=True)
                rec = rp.tile([128, 512], f32, tag="rec", name="rec")
                nc.vector.reciprocal_approx_fast(rec[:], dbc[:])
                # attention @ v, fused normalize on the psum->sbuf move
                o2 = pst.tile([128, 512], f32, tag="pst", name="o2")
                for hh in range(2):
                    h = 2 * g + hh
                    c = hh * 256
                    e0 = slice(hh * 384, hh * 384 + 256)
                    e1 = slice(hh * 384 + 256, hh * 384 + 384)
                    v0 = vsb[:, b * 2, h * 128:(h + 1) * 128]
                    v1 = vsb[:, b * 2 + 1, h * 128:(h + 1) * 128]
                    nc.tensor.matmul(o2[:, c:c + 256], v0, E2[:, e0],
                                     start=(hh == 0), stop=False,
                                     skip_group_check=True)
                    nc.tensor.matmul(o2[:, c + 128:c + 256], v1, E2[:, e1],
                                     start=False, stop=(hh == 1),
                                     skip_group_check=True)
                nc.vector.tensor_tensor(
                    aosb[:, 2 * g:2 * g + 2, b, :],
                    o2[:].rearrange("p (h q) -> p h q", h=2),
                    rec[:].rearrange("p (h q) -> p h q", h=2),
                    mybir.AluOpType.mult)

            # software pipeline, finely zipped: attention chunks of pair N-1
            # are interleaved between projection chunks of pair N so every
            # engine's (in-order) stream always has ready work nearby.
            # Attention is software-pipelined at TWO levels: qkv of pair
            # N-1 is consumed during pair N's projections, and within the
            # attention itself the scores/exp stage (A) of chain j runs one
            # slot ahead of the denominator/AV/normalize tail (B) of chain
            # j-1, so the scalar-exp -> PE -> DVE round trips of adjacent
            # chains overlap.
            prev_qkv = None
            prev_ao = None
            pend = []
            for pr in range(n_pairs):
                have_att = prev_qkv is not None
                aosb = (aop.tile([128, 6, 2, 256], bf16, tag="aosb",
                                 name="aosb") if have_att else None)

                def att(i):
                    if not have_att:
                        return
                    b, g = divmod(i, 3)
                    if g == 0 and prev_ao is not None:
                        emit_outproj(prev_ao[0], prev_ao[1],
                                     range(3 * b, 3 * b + 3))
                    E2 = emit_att_scores(prev_qkv, b, g)
                    emit_att_tail(prev_qkv, aosb, b, g, E2)

                tok = slice(pr * 512, (pr + 1) * 512)
                xt = xp.tile([128, 6, 512], bf16, tag="xt", name="xt")
                nc.sync.dma_start(xt[:],
                                  xT[:, :, tok].rearrange("k p t -> p k t"))
                xr_ql = emit_proj1_slice(xt, qA_s[:, :, 0:128], "ql")
                att(0)
                xr_kl = emit_proj1_slice(xt, kA_s[:, :, 0:128], "kl")
                att(1)
                xr_hi = emit_proj1_slice(xt, qkAh_s, "hi")
                xr_q = (xr_ql, xr_hi)
                xr_k = (xr_kl, xr_hi)
                xr_v = emit_proj1(xt, vA_s, "v")
                att(2)
                qsb = qkp.tile([128, 3072], bf16, tag="qsb", name="qsb")
                ksb = qkp.tile([128, 3072], bf16, tag="ksb", name="ksb")
                for g in range(3):
                    emit_proj2_qk2(g, xr_q, xr_k, qsb, ksb)
                    att(3 + g)
                vsb = emit_vproj(xr_v)
                if have_att:
                    prev_ao = (aosb, pr - 1)
                prev_qkv = (qsb, ksb, vsb)

            # tail: attention for the last pair
            aosb = aop.tile([128, 6, 2, 256], bf16, tag="aosb", name="aosb")
            for b in range(2):
                if prev_ao is not None:
                    emit_outproj(prev_ao[0], prev_ao[1],
                                 range(3 * b, 3 * b + 3))
                for g in range(3):
                    E2 = emit_att_scores(prev_qkv, b, g)
                    emit_att_tail(prev_qkv, aosb, b, g, E2)
            prev_ao = (aosb, n_pairs - 1)
            emit_outproj(prev_ao[0], prev_ao[1], range(6), half=0)
            emit_outproj(prev_ao[0], prev_ao[1], range(6), half=1)

    nc.compile()
    return nc


def _rope_tables():
    inv = 1.0 / (10000.0 ** (np.arange(0, HD, 2, dtype=np.float32) / HD))
    t = np.arange(T, dtype=np.float32)
    freqs = np.outer(t, inv)                      # [T, 64]
    emb = np.concatenate([freqs, freqs], axis=-1)  # [T, 128]
    return np.cos(emb).astype(np.float32), np.sin(emb).astype(np.float32)


def _prep_shared(qA, qB, kA, kB, vA, vB, o_w):
    """Host-side weight/constant layouts (shared by all cores)."""
    def a_layout(A):  # [768,192] -> [6,128,192]
        return _to_bf16(np.ascontiguousarray(A.reshape(6, 128, RANK)))

    def b_layout(Bm):  # [192,768] -> overlapped [2,128,768] (v path)
        Bp = np.zeros((2, 128, D), np.float32)
        Bp[0, 0:64] = Bm[0:64]
        Bp[1] = Bm[64:192]
        return _to_bf16(np.ascontiguousarray(Bp))

    def b_layout_qk(Bm, tail0):  # exact split; tail at parts [tail0:tail0+64]
        Bp = np.zeros((2, 128, D), np.float32)
        Bp[0] = Bm[0:128]
        Bp[1, tail0:tail0 + 64] = Bm[128:192]
        return _to_bf16(np.ascontiguousarray(Bp))

    cos, sin = _rope_tables()
    cosT = np.ascontiguousarray(cos.T)  # [128, 256]
    sinT = np.ascontiguousarray(sin.T)
    cos2 = np.concatenate([cosT, cosT], axis=1)  # [128, 512] (2 batch items)
    sinsg2 = np.concatenate([sinT, sinT], axis=1).copy()
    sinsg2[0:64] = -sinsg2[0:64]   # mrot[p<64] = msb[p+64] pairs with -sin
    cos4 = np.concatenate([cos2, cos2], axis=1)   # [128, 1024] head pair
    sin4 = np.concatenate([sinsg2, sinsg2], axis=1)

    # additive causal mask: the two -1e4 triangles (key-tile0 vs q 0:128,
    # key-tile1 vs q 128:256 -- identical patterns), stored adjacently
    p = np.arange(128)[:, None]
    c1 = np.arange(128)[None, :]
    tri1 = np.where(p > c1, -10000.0, 0.0).astype(np.float32)
    tri = np.concatenate([tri1, tri1], axis=1)  # [128, 256]

    return {
        "qA_l": a_layout(qA), "kA_l": a_layout(kA), "vA_l": a_layout(vA),
        "qkAh_l": _to_bf16(np.ascontiguousarray(np.concatenate(
            [qA.reshape(6, 128, RANK)[:, :, 128:192],
             kA.reshape(6, 128, RANK)[:, :, 128:192]], axis=2))),
        "qB_l": b_layout_qk(qB, 0), "kB_l": b_layout_qk(kB, 64),
        "vB_l": b_layout(vB),
        "ow_l": _to_bf16(np.ascontiguousarray(o_w.reshape(6, 128, D))),
        "cos4": _to_bf16(cos4), "sin4": _to_bf16(sin4),
        "tri_m": _to_bf16(tri),
        "eye_m": _to_bf16(np.eye(128, dtype=np.float32)),
        "ones_m": _to_bf16(np.ones((128, 128), np.float32)),
    }


def x_to_xT(xc):
    """[b, T, D] -> [6, 128, b*T] feature-major, batch-major tokens."""
    nb = xc.shape[0]
    return _to_bf16(np.ascontiguousarray(
        xc.reshape(nb, T, 6, 128).transpose(2, 3, 0, 1).reshape(6, 128, nb * T)))


def outT_to_out(oT, nb):
    return np.ascontiguousarray(
        oT.astype(np.float32).reshape(6, 128, nb, T)
        .transpose(2, 3, 0, 1).reshape(nb, T, D))


def kernel(x, qA, qB, kA, kB, vA, vB, o_w):
    from concourse import bass_utils

    if "nc" not in _CACHE:
        _CACHE["nc"] = build_program(N_PAIRS)
    nc = _CACHE["nc"]

    shared = _prep_shared(
        np.asarray(qA, np.float32), np.asarray(qB, np.float32),
        np.asarray(kA, np.float32), np.asarray(kB, np.float32),
        np.asarray(vA, np.float32), np.asarray(vB, np.float32),
        np.asarray(o_w, np.float32))
    x = np.asarray(x, np.float32)

    in_maps = []
    for c in range(N_CORES):
        m = dict(shared)
        m["xT"] = x_to_xT(x[c * B_LOC:(c + 1) * B_LOC])
        in_maps.append(m)

    res = bass_utils.run_bass_kernel_spmd(
        nc, in_maps, core_ids=list(range(N_CORES)))
    out = np.empty((B, T, D), np.float32)
    for c in range(N_CORES):
        out[c * B_LOC:(c + 1) * B_LOC] = outT_to_out(
            res.results[c]["outT"], B_LOC)
    return out


# revision 11
# speedup vs baseline: 1.0810x; 1.0810x over previous
"""Trainium2 Bass kernel for nn_Attn_30623116820602.

Low-rank-projected causal multi-head attention:
  q/k/v = (x @ A) @ B  (rank 192), RoPE on q,k, causal softmax attention,
  output projection.  x: [128, 256, 768] fp32.

Sharding: pure data-parallel over batch (16 items per core, 8 cores).
Feature-major layout on device (d_model on partitions); host pre-transposes
x per core and post-transposes the result.

v4: bf16 matmul operands everywhere (psum stays f32).
  - RoPE emitted per head-PAIR on [128,1024] tiles: psum->sbuf copies on
    ScalarE, rotate-half partition swap via two Sync-queue DMAs, the two
    multiplies on DVE (bf16 2x mode), the add on GpSimd.
  - Causal mask: exp runs unmasked; the two causal triangles of each E tile
    are then zeroed by one strided DVE multiply per (b,g) against a resident
    0/1 triangle table.  (Cheaper than the old identity-stationary -1e4
    matmul: N<=256 matmuls are LDWEIGHTS-bound on this stack.)
  - Both heads' exp outputs share one E2 [128,768] tile, so the softmax
    denominators take two wide strided matmuls instead of four narrow ones.
  - Denominators broadcast across partitions by an all-ones stationary;
    normalization fused into the PSUM->SBUF move of the attention output.
"""

import math
import sys

sys.path.insert(0, "/opt/trn_rl_repo")

import numpy as np
import ml_dtypes


def _to_bf16(a):
    return a.astype(ml_dtypes.bfloat16)


B, T, D = 128, 256, 768
H, HD = 6, 128
RANK = 192  # padded to 256 on device
N_CORES = 8
B_LOC = B // N_CORES  # 16
N_PAIRS = B_LOC // 2  # 8 (2 batch items per pipeline iteration)
SCALE = 1.0 / math.sqrt(HD)

_CACHE = {}


def build_program(n_pairs=N_PAIRS):
    import concourse.tile as tile
    from concourse import bacc, mybir
    from contextlib import ExitStack

    f32 = mybir.dt.float32
    bf16 = mybir.dt.bfloat16
    TOK = n_pairs * 512

    nc = bacc.Bacc("TRN2", target_bir_lowering=False, debug=False,
                   num_devices=N_CORES)

    def din(name, shape, dt=bf16):
        return nc.dram_tensor(name, shape, dt, kind="ExternalInput").ap()

    xT = din("xT", [6, 128, TOK])
    qA_l, kA_l, vA_l = (din(n, [6, 128, 192]) for n in ("qA_l", "kA_l", "vA_l"))
    qkAh_l = din("qkAh_l", [6, 128, 128])  # [qA_hi | kA_hi] fused tails
    qB_l, kB_l, vB_l = (din(n, [2, 128, 768]) for n in ("qB_l", "kB_l", "vB_l"))
    ow_l = din("ow_l", [6, 128, 768])
    cos4 = din("cos4", [128, 1024])
    sin4 = din("sin4", [128, 1024])
    tri_m = din("tri_m", [128, 256])     # 0 / -1e4 triangle pair
    eye_m = din("eye_m", [128, 128])     # identity
    ones_m = din("ones_m", [128, 128])   # all ones
    outT = nc.dram_tensor("outT", [6, 128, TOK], bf16,
                          kind="ExternalOutput").ap()

    with tile.TileContext(nc) as tc:
        with ExitStack() as ctx:
            wp = ctx.enter_context(tc.tile_pool(name="w", bufs=1))
            xp = ctx.enter_context(tc.tile_pool(name="xt", bufs=2))
            xrp = ctx.enter_context(tc.tile_pool(name="xr", bufs=2))
            mp = ctx.enter_context(tc.tile_pool(name="msb", bufs=3))
            qkp = ctx.enter_context(tc.tile_pool(name="qk", bufs=2))
            ep = ctx.enter_context(tc.tile_pool(name="eexp", bufs=6))
            rp = ctx.enter_context(tc.tile_pool(name="rec", bufs=3))
            aop = ctx.enter_context(tc.tile_pool(name="ao", bufs=2))
            fp = ctx.enter_context(tc.tile_pool(name="fout", bufs=2))
            ps = ctx.enter_context(tc.tile_pool(name="ps", bufs=4, space="PSUM"))
            psa = ctx.enter_context(tc.tile_pool(name="psa", bufs=2, space="PSUM"))
            psb = ctx.enter_context(tc.tile_pool(name="psb", bufs=2, space="PSUM"))

            # ---- resident weights / constants (all bf16) ----
            def wload(name, src, shape, perm=None):
                t = wp.tile(shape, bf16, tag=name, name=name)
                s = src if perm is None else src.rearrange(perm)
                nc.gpsimd.dma_start(t[:], s)
                return t

            qA_s = wload("qA", qA_l, [128, 6, 192], "k p m -> p k m")
            kA_s = wload("kA", kA_l, [128, 6, 192], "k p m -> p k m")
            vA_s = wload("vA", vA_l, [128, 6, 192], "k p m -> p k m")
            qkAh_s = wload("qkAh", qkAh_l, [128, 6, 128], "k p m -> p k m")
            qB_s = wload("qB", qB_l, [128, 2, 768], "k p m -> p k m")
            kB_s = wload("kB", kB_l, [128, 2, 768], "k p m -> p k m")
            vB_s = wload("vB", vB_l, [128, 2, 768], "k p m -> p k m")
            ow_s = wload("ow", ow_l, [128, 6, 768], "k p m -> p k m")
            cos_s = wload("cos", cos4, [128, 1024])
            sin_s = wload("sin", sin4, [128, 1024])
            tri_s = wload("tri", tri_m, [128, 256])
            eye_s = wload("eye", eye_m, [128, 128])
            ones_s = wload("ones", ones_m, [128, 128])

            def emit_outproj(aosb_prev, pr_prev, mts, half=None):
                # half=None: both batch items (512 tok); half=b: one item
                w = 512 if half is None else 256
                c0 = 0 if half in (None, 0) else 256
                tokp = slice(pr_prev * 512 + c0, pr_prev * 512 + c0 + w)
                for mt in mts:
                    fps = ps.tile([128, 512], f32, tag="ps", name="fps")
                    for kt in range(6):
                        mov = (aosb_prev[:, kt, :, :] if half is None
                               else aosb_prev[:, kt, half, :])
                        nc.tensor.matmul(
                            fps[:, 0:w],
                            ow_s[:, kt, mt * 128:(mt + 1) * 128],
                            mov,
                            start=(kt == 0), stop=(kt == 5))
                    fout = fp.tile([128, 512], bf16, tag="fout", name="fout")
                    nc.vector.tensor_copy(fout[:, 0:w], fps[:, 0:w])
                    nc.sync.dma_start(outT[mt, :, tokp], fout[:, 0:w])

            def emit_proj1(xt, A_s, pname):
                mm = [psb.tile([128, 512], f32, tag="psb", name="p1"),
                      psb.tile([128, 512], f32, tag="psb", name="p1")]
                for mt in range(2):
                    for kt in range(6):
                        nc.tensor.matmul(
                            mm[mt][:],
                            A_s[:, kt, mt * 64:mt * 64 + 128],
                            xt[:, kt, :],
                            start=(kt == 0), stop=(kt == 5))
                xr = xrp.tile([128, 2, 512], bf16, tag=f"xr_{pname}",
                              name=f"xr_{pname}")
                nc.scalar.copy(xr[:, 0, :], mm[0][:])
                nc.scalar.copy(xr[:, 1, :], mm[1][:])
                return xr

            def emit_proj1_slice(xt, A_s, pname):
                # one 128-col stationary slice -> one [128,512] xr tile
                pm = psb.tile([128, 512], f32, tag="psb", name=f"p1{pname}")
                for kt in range(6):
                    nc.tensor.matmul(
                        pm[:], A_s[:, kt, :], xt[:, kt, :],
                        start=(kt == 0), stop=(kt == 5))
                xr = xrp.tile([128, 512], bf16, tag=f"xr_{pname}",
                              name=f"xr_{pname}")
                nc.scalar.copy(xr[:], pm[:])
                return xr

            def emit_proj2_qk2(g, xr_q, xr_k, qsb, ksb):
                # proj2 + RoPE for heads 2g, 2g+1 of q and k, on [128,1024]
                # head-pair tiles (halves the elementwise instruction count).
                for name, xr, B_s, sb in (("q", xr_q, qB_s, qsb),
                                          ("k", xr_k, kB_s, ksb)):
                    msb = mp.tile([128, 1024], bf16, tag="msb", name="msb")
                    for hh in range(2):
                        h = 2 * g + hh
                        hsl = slice(h * 128, (h + 1) * 128)
                        pm = psa.tile([128, 512], f32, tag="psa", name="pm")
                        for kt in range(2):
                            nc.tensor.matmul(
                                pm[:], B_s[:, kt, hsl], xr[kt][:],
                                start=(kt == 0), stop=(kt == 1))
                        nc.scalar.copy(msb[:, hh * 512:(hh + 1) * 512], pm[:])
                    # rotate-half: partition swap via DMA, aligned DVE
                    # multiplies (bf16 2x), add on GpSimd
                    mrot = mp.tile([128, 1024], bf16, tag="mrot", name="mrot")
                    nc.sync.dma_start(mrot[0:64, :], msb[64:128, :])
                    nc.sync.dma_start(mrot[64:128, :], msb[0:64, :])
                    trot = mp.tile([128, 1024], bf16, tag="trot", name="trot")
                    hs = slice(2 * g * 512, 2 * g * 512 + 1024)
                    nc.vector.tensor_tensor(
                        sb[:, hs], msb[:], cos_s[:], mybir.AluOpType.mult)
                    nc.vector.tensor_tensor(
                        trot[:], mrot[:], sin_s[:], mybir.AluOpType.mult)
                    nc.gpsimd.tensor_tensor(
                        sb[:, hs], sb[:, hs], trot[:], mybir.AluOpType.add)

            def emit_vproj(xrv):
                vsb = qkp.tile([128, 4, 768], bf16, tag="vsb", name="vsb")
                for mt in range(4):
                    for nch in range(2):
                        vp = psb.tile([128, 512], f32, tag="psb", name="vp")
                        for kt in range(2):
                            nc.tensor.matmul(
                                vp[:, 0:384],
                                xrv[:, kt, mt * 128:(mt + 1) * 128],
                                vB_s[:, kt, nch * 384:(nch + 1) * 384],
                                start=(kt == 0), stop=(kt == 1))
                        nc.scalar.copy(vsb[:, mt, nch * 384:(nch + 1) * 384],
                                       vp[:, 0:384])
                return vsb

            def emit_att_scores(qkv, b, g):
                # E2 layout per (b, g): [128 keys, 768]; per head hh half:
                #   cols 0:256   = key-tile 0 vs q 0:256
                #   cols 256:384 = key-tile 1 vs q 128:256
                qsb, ksb, vsb = qkv
                E2 = ep.tile([128, 768], bf16, tag="E", name="E")
                del vsb
                for hh in range(2):
                    h = 2 * g + hh
                    qall = slice(h * 512 + b * 256, h * 512 + b * 256 + 256)
                    qhi = slice(h * 512 + b * 256 + 128,
                                h * 512 + b * 256 + 256)
                    k0 = slice(h * 512 + b * 256, h * 512 + b * 256 + 128)
                    k1 = slice(h * 512 + b * 256 + 128,
                               h * 512 + b * 256 + 256)
                    sp = ps.tile([128, 512], f32, tag="ps", name="sp")
                    nc.tensor.matmul(sp[:, 0:256], ksb[:, k0], qsb[:, qall],
                                     start=True, stop=False,
                                     skip_group_check=True)
                    nc.tensor.matmul(sp[:, 256:384], ksb[:, k1], qsb[:, qhi],
                                     start=False, stop=False,
                                     skip_group_check=True)
                    # one matmul adds -1e4 to both causal triangles
                    # (cols 0:128 and 256:384) via a strided psum AP
                    nc.tensor.matmul(
                        sp[:].rearrange("p (a b) -> p a b", a=4)[:, 0:3:2, :],
                        eye_s[:], tri_s[:],
                        start=False, stop=True, skip_group_check=True)
                    nc.scalar.activation(E2[:, hh * 384:hh * 384 + 384],
                                         sp[:, 0:384],
                                         mybir.ActivationFunctionType.Exp,
                                         scale=SCALE)
                return E2

            def emit_att_tail(qkv, aosb, b, g, E2):
                qsb, ksb, vsb = qkv
                # denominators, broadcast across partitions by the all-ones
                # stationary; two wide strided matmuls over both heads.
                dbc = ps.tile([128, 512], f32, tag="ps", name="dbc")
                for hh in range(2):
                    c = hh * 256
                    e = hh * 384
                    nc.tensor.matmul(dbc[:, c:c + 256], ones_s[:],
                                     E2[:, e:e + 256],
                                     start=(hh == 0), stop=False,
                                     skip_group_check=True)
                    nc.tensor.matmul(dbc[:, c + 128:c + 256], ones_s[:],
                                     E2[:, e + 256:e + 384], start=False,
                                     stop=(hh == 1), skip_group_check=True)
                rec = rp.tile([128, 512], f32, tag="rec", name="rec")
                nc.vector.reciprocal_approx_fast(rec[:], dbc[:])
                # attention @ v, fused normalize on the psum->sbuf move
                o2 = ps.tile([128, 512], f32, tag="ps", name="o2")
                for hh in range(2):
                    h = 2 * g + hh
                    c = hh * 256
                    e0 = slice(hh * 384, hh * 384 + 256)
                    e1 = slice(hh * 384 + 256, hh * 384 + 384)
                    v0 = vsb[:, b * 2, h * 128:(h + 1) * 128]
                    v1 = vsb[:, b * 2 + 1, h * 128:(h + 1) * 128]
                    nc.tensor.matmul(o2[:, c:c + 256], v0, E2[:, e0],
                                     start=(hh == 0), stop=False,
                                     skip_group_check=True)
                    nc.tensor.matmul(o2[:, c + 128:c + 256], v1, E2[:, e1],
                                     start=False, stop=(hh == 1),
                                     skip_group_check=True)
                nc.vector.tensor_tensor(
                    aosb[:, 2 * g:2 * g + 2, b, :],
                    o2[:].rearrange("p (h q) -> p h q", h=2),
                    rec[:].rearrange("p (h q) -> p h q", h=2),
                    mybir.AluOpType.mult)

            # software pipeline, finely zipped: attention chunks of pair N-1
            # are interleaved between projection chunks of pair N so every
            # engine's (in-order) stream always has ready work nearby.
            # Attention is software-pipelined at TWO levels: qkv of pair
            # N-1 is consumed during pair N's projections, and within the
            # attention itself the scores/exp stage (A) of chain j runs one
            # slot ahead of the denominator/AV/normalize tail (B) of chain
            # j-1, so the scalar-exp -> PE -> DVE round trips of adjacent
            # chains overlap.
            prev_qkv = None
            prev_ao = None
            pend = []
            for pr in range(n_pairs):
                have_att = prev_qkv is not None
                aosb = (aop.tile([128, 6, 2, 256], bf16, tag="aosb",
                                 name="aosb") if have_att else None)

                def att(i):
                    if not have_att:
                        return
                    b, g = divmod(i, 3)
                    if g == 0 and prev_ao is not None:
                        emit_outproj(prev_ao[0], prev_ao[1],
                                     range(3 * b, 3 * b + 3))
                    E2 = emit_att_scores(prev_qkv, b, g)
                    pend.append((prev_qkv, aosb, b, g, E2))
                    if len(pend) > 1:
                        emit_att_tail(*pend.pop(0))

                tok = slice(pr * 512, (pr + 1) * 512)
                xt = xp.tile([128, 6, 512], bf16, tag="xt", name="xt")
                nc.sync.dma_start(xt[:],
                                  xT[:, :, tok].rearrange("k p t -> p k t"))
                xr_ql = emit_proj1_slice(xt, qA_s[:, :, 0:128], "ql")
                att(0)
                xr_kl = emit_proj1_slice(xt, kA_s[:, :, 0:128], "kl")
                att(1)
                xr_hi = emit_proj1_slice(xt, qkAh_s, "hi")
                xr_q = (xr_ql, xr_hi)
                xr_k = (xr_kl, xr_hi)
                xr_v = emit_proj1(xt, vA_s, "v")
                att(2)
                qsb = qkp.tile([128, 3072], bf16, tag="qsb", name="qsb")
                ksb = qkp.tile([128, 3072], bf16, tag="ksb", name="ksb")
                for g in range(3):
                    emit_proj2_qk2(g, xr_q, xr_k, qsb, ksb)
                    att(3 + g)
                vsb = emit_vproj(xr_v)
                while pend:
                    emit_att_tail(*pend.pop(0))
                if have_att:
                    prev_ao = (aosb, pr - 1)
                prev_qkv = (qsb, ksb, vsb)

            # tail: attention for the last pair
            aosb = aop.tile([128, 6, 2, 256], bf16, tag="aosb", name="aosb")
            for b in range(2):
                if prev_ao is not None:
                    emit_outproj(prev_ao[0], prev_ao[1],
                                 range(3 * b, 3 * b + 3))
                for g in range(3):
                    E2 = emit_att_scores(prev_qkv, b, g)
                    pend.append((prev_qkv, aosb, b, g, E2))
                    if len(pend) > 1:
                        emit_att_tail(*pend.pop(0))
            while pend:
                emit_att_tail(*pend.pop(0))
            prev_ao = (aosb, n_pairs - 1)
            emit_outproj(prev_ao[0], prev_ao[1], range(6), half=0)
            emit_outproj(prev_ao[0], prev_ao[1], range(6), half=1)

    nc.compile()
    return nc


def _rope_tables():
    inv = 1.0 / (10000.0 ** (np.arange(0, HD, 2, dtype=np.float32) / HD))
    t = np.arange(T, dtype=np.float32)
    freqs = np.outer(t, inv)                      # [T, 64]
    emb = np.concatenate([freqs, freqs], axis=-1)  # [T, 128]
    return np.cos(emb).astype(np.float32), np.sin(emb).astype(np.float32)


def _prep_shared(qA, qB, kA, kB, vA, vB, o_w):
    """Host-side weight/constant layouts (shared by all cores)."""
    def a_layout(A):  # [768,192] -> [6,128,192]
        return _to_bf16(np.ascontiguousarray(A.reshape(6, 128, RANK)))

    def b_layout(Bm):  # [192,768] -> overlapped [2,128,768] (v path)
        Bp = np.zeros((2, 128, D), np.float32)
        Bp[0, 0:64] = Bm[0:64]
        Bp[1] = Bm[64:192]
        return _to_bf16(np.ascontiguousarray(Bp))

    def b_layout_qk(Bm, tail0):  # exact split; tail at parts [tail0:tail0+64]
        Bp = np.zeros((2, 128, D), np.float32)
        Bp[0] = Bm[0:128]
        Bp[1, tail0:tail0 + 64] = Bm[128:192]
        return _to_bf16(np.ascontiguousarray(Bp))

    cos, sin = _rope_tables()
    cosT = np.ascontiguousarray(cos.T)  # [128, 256]
    sinT = np.ascontiguousarray(sin.T)
    cos2 = np.concatenate([cosT, cosT], axis=1)  # [128, 512] (2 batch items)
    sinsg2 = np.concatenate([sinT, sinT], axis=1).copy()
    sinsg2[0:64] = -sinsg2[0:64]   # mrot[p<64] = msb[p+64] pairs with -sin
    cos4 = np.concatenate([cos2, cos2], axis=1)   # [128, 1024] head pair
    sin4 = np.concatenate([sinsg2, sinsg2], axis=1)

    # additive causal mask: the two -1e4 triangles (key-tile0 vs q 0:128,
    # key-tile1 vs q 128:256 -- identical patterns), stored adjacently
    p = np.arange(128)[:, None]
    c1 = np.arange(128)[None, :]
    tri1 = np.where(p > c1, -10000.0, 0.0).astype(np.float32)
    tri = np.concatenate([tri1, tri1], axis=1)  # [128, 256]

    return {
        "qA_l": a_layout(qA), "kA_l": a_layout(kA), "vA_l": a_layout(vA),
        "qkAh_l": _to_bf16(np.ascontiguousarray(np.concatenate(
            [qA.reshape(6, 128, RANK)[:, :, 128:192],
             kA.reshape(6, 128, RANK)[:, :, 128:192]], axis=2))),
        "qB_l": b_layout_qk(qB, 0), "kB_l": b_layout_qk(kB, 64),
        "vB_l": b_layout(vB),
        "ow_l": _to_bf16(np.ascontiguousarray(o_w.reshape(6, 128, D))),
        "cos4": _to_bf16(cos4), "sin4": _to_bf16(sin4),
        "tri_m": _to_bf16(tri),
        "eye_m": _to_bf16(np.eye(128, dtype=np.float32)),
        "ones_m": _to_bf16(np.ones((128, 128), np.float32)),
    }


def x_to_xT(xc):
    """[b, T, D] -> [6, 128, b*T] feature-major, batch-major tokens."""
    nb = xc.shape[0]
    return _to_bf16(np.ascontiguousarray(
        xc.reshape(nb, T, 6, 128).transpose(2, 3, 0, 1).reshape(6, 128, nb * T)))


def outT_to_out(oT, nb):
    return np.ascontiguousarray(
        oT.astype(np.float32).reshape(6, 128, nb, T)
        .transpose(2, 3, 0, 1).reshape(nb, T, D))


def kernel(x, qA, qB, kA, kB, vA, vB, o_w):
    from concourse import bass_utils

    if "nc" not in _CACHE:
        _CACHE["nc"] = build_program(N_PAIRS)
    nc = _CACHE["nc"]

    shared = _prep_shared(
        np.asarray(qA, np.float32), np.asarray(qB, np.float32),
        np.asarray(kA, np.float32), np.asarray(kB, np.float32),
        np.asarray(vA, np.float32), np.asarray(vB, np.float32),
        np.asarray(o_w, np.float32))
    x = np.asarray(x, np.float32)

    in_maps = []
    for c in range(N_CORES):
        m = dict(shared)
        m["xT"] = x_to_xT(x[c * B_LOC:(c + 1) * B_LOC])
        in_maps.append(m)

    res = bass_utils.run_bass_kernel_spmd(
        nc, in_maps, core_ids=list(range(N_CORES)))
    out = np.empty((B, T, D), np.float32)
    for c in range(N_CORES):
        out[c * B_LOC:(c + 1) * B_LOC] = outT_to_out(
            res.results[c]["outT"], B_LOC)
    return out
